# revision 6
# baseline (speedup 1.0000x reference)
"""Trainium2 Bass kernel for nn_CDGMLinear (2-layer graph-learning GNN), v3.

Structure (per core, SPMD over 8 cores; row-block sharding of the N x N
affinity):
  * chunk-major main loop: for each i-chunk (1024 of the core's 2048 rows),
    stream all 128 j-tiles: z = aug_g^T aug_mov (PE, bf16) -> sigmoid (ACT)
    -> msg += gnn_t @ adj (PE, PSUM-accum) + rowsum += adj (DVE, bf16).
  * per-chunk normalize + AllGather so the collective hides under the next
    chunk's N^2 work.
  * global prep (aug_g / sqb / gnn_t over all N) is split into 4 j-groups
    with per-group tiles; layer-2 groups depend only on their AllGather
    chunk, so group prep and the early j-tiles of layer 2 overlap the
    second AllGather.  Layer-2 gathered x uses an ic-major column order
    (a permutation of j, which the math is invariant to).
  * local prep (aug_mov / corr, from the core's own rows) uses per-chunk
    tiles emitted right after each chunk's normalize.
  * output head deferred to the end (single ACT table switch to Exp).

Precision: all O(N^2) matmuls bf16; fp32 diagonal correction via corr
(msg gets sigth*gnn_f32 - bf16(sigth)*gnn_bf16); bf16 rowsum accumulator
(error ~1e-3 relative, tolerance 2e-2).
"""
import numpy as np
import ml_dtypes

import concourse.bass as bass
import concourse.bacc as bacc
import concourse.tile as tile
import concourse.mybir as mybir
from concourse.bass_utils import run_bass_kernel_spmd

F32 = mybir.dt.float32
BF16 = mybir.dt.bfloat16
Act = mybir.ActivationFunctionType
Alu = mybir.AluOpType
AX = mybir.AxisListType.X

N = 16384
D = 128
L = 64
NCORES = 8
B = N // NCORES          # 2048 rows per core
JT = N // 128            # 128 j-tiles
ICH = 1024               # i-chunk width of the main loop
NIC = B // ICH           # 2 chunks
NG = 4                   # j-groups for global prep
GW = N // NG             # group width (4096 columns)
GJT = JT // NG           # j-tiles per group (32)
NOUT = 10

_NC_CACHE = {}


def _prep_local_aug(nc, sb, pp, st_, w, lidx, ic, use_act=True):
    """Block-local prep part 1 for chunk ic: the aug_mov moving operand
    (rows 0:64 bf16 2t*g, rows 64:66 hi/lo of -sq_i/2) and gr/gsqr."""
    if ic == 0:
        bcb = pp.tile([128, 512], F32, name=f"bcb{lidx}", tag="pz")
        for q in range(4):
            nc.tensor.matmul(bcb[:, q * 128:(q + 1) * 128], w["ones1f"][:],
                             w["gnnbrow"][:, :], start=True, stop=True)
        bcb_sb = sb.tile([128, 512], F32, name=f"bcb_sb{lidx}", tag="bcb_sb")
        nc.vector.tensor_copy(bcb_sb[:], bcb[:])
        st_["bcb_sb"] = bcb_sb
        st_["aug_mov"] = {}
        st_["gr"] = {}
        st_["corr"] = {}
    xr_bf = st_["xr_bf"][ic]

    aug_mov = sb.tile([66, ICH], BF16, name=f"aug_mov{lidx}_{ic}",
                      tag=f"aug_mov{ic}")
    gr = sb.tile([64, ICH], BF16, name=f"gr{lidx}_{ic}", tag=f"gr{ic}")
    gsqr = sb.tile([64, ICH], F32, name=f"gsqr{lidx}_{ic}", tag=f"gsqr{ic}")
    for bc in range(ICH // 512):
        cs = slice(bc * 512, (bc + 1) * 512)
        gp3 = pp.tile([64, 512], F32, name=f"gp3{lidx}_{ic}_{bc}", tag="pz")
        nc.tensor.matmul(gp3[:], w["wgl_bf"][:], xr_bf[:, cs],
                         start=True, stop=True)
        if use_act and bc % 2 == 0:
            nc.scalar.activation(gr[:, cs], gp3[:], Act.Relu,
                                 bias=w["glb"][0:64, :])
        else:
            nc.vector.tensor_scalar(gr[:, cs], gp3[:], w["glb"][0:64, :], 0.0,
                                    Alu.add, Alu.max)
        nc.vector.tensor_scalar_mul(aug_mov[0:64, cs], gr[:, cs],
                                    w["twot"][0:64, :])
        # exactly the products the PE's diagonal contraction computes
        nc.vector.tensor_tensor(gsqr[:, cs], gr[:, cs], aug_mov[0:64, cs],
                                Alu.mult)
    # sq_i row: -sq_i/2 as hi/lo bf16 pair (rows 64, 65)
    for bc in range(ICH // 512):
        cs = slice(bc * 512, (bc + 1) * 512)
        sqi = pp.tile([1, 512], F32, name=f"sqi{lidx}_{ic}_{bc}", tag="pz")
        for h in range(2):
            nc.tensor.matmul(sqi[:, h * 256:(h + 1) * 256], w["ones64f"][:],
                             gsqr[0:64, bc * 512 + h * 256: bc * 512 + (h + 1) * 256],
                             start=True, stop=True)
        nsq = sb.tile([1, 512], F32, name=f"nsq{lidx}_{ic}_{bc}", tag="nsq")
        nc.vector.tensor_scalar_mul(nsq[:], sqi[:], -0.5)
        hi = sb.tile([1, 512], BF16, name=f"hi{lidx}_{ic}_{bc}", tag="hi")
        nc.vector.tensor_copy(hi[:], nsq[:])
        lo = sb.tile([1, 512], F32, name=f"lo{lidx}_{ic}_{bc}", tag="lo")
        nc.vector.tensor_tensor(lo[:], nsq[:], hi[:], Alu.subtract)
        lob = sb.tile([1, 512], BF16, name=f"lob{lidx}_{ic}_{bc}", tag="lob")
        nc.vector.tensor_copy(lob[:], lo[:])
        nc.sync.dma_start(aug_mov[64:65, cs], hi[:])
        nc.sync.dma_start(aug_mov[65:66, cs], lob[:])
    st_["aug_mov"][ic] = aug_mov
    st_["gr"][ic] = gr


def _prep_local_corr(nc, sb, pp, st_, w, lidx, ic, half=None):
    """Block-local prep part 2 for chunk ic: the fp32 diagonal correction
    corr[f, i] = sigth * gnn_f32[f,i] - bf16(sigth) * gnn_bf16_stored[f,i].
    half=0/1 emits only that 512-column half (to soften prep bursts)."""
    bcb_sb = st_["bcb_sb"]
    xr_bf, xr_f32 = st_["xr_bf"][ic], st_["xr_f32"][ic]
    if half in (None, 0):
        corr = sb.tile([128, ICH], F32, name=f"corr{lidx}_{ic}",
                       tag=f"corr{ic}")
        st_["corr"][ic] = corr
    else:
        corr = st_["corr"][ic]
    bcs = range(ICH // 512) if half is None else [half]
    for bc in bcs:
        cs = slice(bc * 512, (bc + 1) * 512)
        gt = pp.tile([128, 512], F32, name=f"gt{lidx}_{ic}_{bc}", tag="pz")
        for h in range(2):
            nc.tensor.matmul(gt[:, h * 256:(h + 1) * 256], w["wgn_f32"][:],
                             xr_f32[:, bc * 512 + h * 256: bc * 512 + (h + 1) * 256],
                             start=True, stop=True)
        nc.vector.tensor_scalar(corr[:, cs], gt[:], w["wgnb"][:],
                                w["sigthv"][:], Alu.add, Alu.mult)
    # reproduce the bf16 stored gnn values for the block, transpose, subtract
    bts = range(ICH // 128) if half is None else range(half * 4, half * 4 + 4)
    for bt in bts:
        grp, q = bt // 4, bt % 4
        if q == 0:
            gp4 = pp.tile([128, 512], F32, name=f"gp4{lidx}_{ic}_{grp}",
                          tag="pz")
            st = sb.tile([128, 512], BF16, name=f"st{lidx}_{ic}_{grp}",
                         tag="st", bufs=2)
        nc.tensor.matmul(gp4[:, q * 128:(q + 1) * 128],
                         xr_bf[:, bt * 128:(bt + 1) * 128],
                         w["wgn_bf"][:], start=True, stop=True)
        if q == 3:
            nc.vector.tensor_tensor(st[:], gp4[:], bcb_sb[:], Alu.add)
            for qq in range(4):
                bt2 = grp * 4 + qq
                tp = pp.tile([128, 128], BF16, name=f"tp{lidx}_{ic}_{bt2}",
                             tag="pz")
                nc.tensor.transpose(tp[:], st[:, qq * 128:(qq + 1) * 128],
                                    w["ident"][:])
                st2 = sb.tile([128, 128], F32, name=f"st2{lidx}_{ic}_{bt2}",
                              tag="st2", bufs=2)
                nc.vector.tensor_scalar_mul(st2[:], tp[:], w["bfsigthv"][:])
                nc.vector.tensor_tensor(corr[:, bt2 * 128:(bt2 + 1) * 128],
                                        corr[:, bt2 * 128:(bt2 + 1) * 128],
                                        st2[:], Alu.subtract)


def _prep_global_alloc(nc, sb, st_, w, lidx, g):
    """Allocate the aug_g tile for group g and emit its ones-rows DMA (early
    SP-queue position; the compute is emitted separately)."""
    if g == 0:
        st_["aug_g"] = {}
        st_["sqb"] = {}
        st_["gnn_t"] = {}
    aug_g = sb.tile([66, GW], BF16, name=f"aug_g{lidx}_{g}", tag=f"aug_g{g}")
    nc.sync.dma_start(aug_g[64:66, :], w["ones2"][:, :])
    st_["aug_g"][g] = aug_g


def _prep_global_group(nc, sb, pp, st_, w, lidx, g, x_g, use_act=True,
                       stages=(0, 1)):
    """Full-N prep compute for j-group g: aug_g rows 0:64, sqb, gnn_t from
    the [D, GW] activation tile x_g.  use_act=False keeps all elementwise
    work off the ACT engine (for prep that runs inside the sigmoid stream).
    stages: 0 = aug_g + sqb, 1 = gnn_t (can be emitted separately)."""
    bcb_sb = st_["bcb_sb"]
    aug_g = st_["aug_g"][g]
    if 0 not in stages:
        return _prep_global_gnn(nc, sb, pp, st_, w, lidx, g, x_g)
    for jc in range(GW // 512):
        gp = pp.tile([64, 512], F32, name=f"gp{lidx}_{g}_{jc}", tag="pz")
        nc.tensor.matmul(gp[:], w["wgl_bf"][:], x_g[:, jc * 512:(jc + 1) * 512],
                         start=True, stop=True)
        if use_act and jc % 2 == 0:
            nc.scalar.activation(aug_g[0:64, jc * 512:(jc + 1) * 512], gp[:],
                                 Act.Relu, bias=w["glb"][0:64, :])
        else:
            nc.vector.tensor_scalar(aug_g[0:64, jc * 512:(jc + 1) * 512],
                                    gp[:], w["glb"][0:64, :], 0.0,
                                    Alu.add, Alu.max)

    # sqb[j_local, jt] = th - t * sq_j   (f32), streamed in 1024-col pieces
    sqps = pp.tile([128, GJT], F32, name=f"sqps{lidx}_{g}", tag="pz")
    for sub in range(GW // 1024):
        gsqb = sb.tile([64, 1024], BF16, name=f"gsqb{lidx}_{g}_{sub}",
                       tag="gsqb", bufs=2)
        for jc in range(2):
            cs = slice(sub * 1024 + jc * 512, sub * 1024 + (jc + 1) * 512)
            if use_act and jc == 0:
                nc.scalar.activation(gsqb[:, jc * 512:(jc + 1) * 512],
                                     aug_g[0:64, cs], Act.Square)
            else:
                nc.vector.tensor_tensor(gsqb[:, jc * 512:(jc + 1) * 512],
                                        aug_g[0:64, cs], aug_g[0:64, cs],
                                        Alu.mult)
        for jl8 in range(8):
            jl = sub * 8 + jl8
            nc.tensor.matmul(sqps[:, jl:jl + 1],
                             gsqb[:, jl8 * 128:(jl8 + 1) * 128],
                             w["ones64b"][:], start=True, stop=True)
    sqb = sb.tile([128, GJT], F32, name=f"sqb{lidx}_{g}", tag=f"sqb{g}")
    nc.vector.tensor_scalar(sqb[:], sqps[:], w["negt"][:], w["thv"][:],
                            Alu.mult, Alu.add)

    st_["sqb"][g] = sqb
    if 1 in stages:
        _prep_global_gnn(nc, sb, pp, st_, w, lidx, g, x_g)


def _prep_global_gnn(nc, sb, pp, st_, w, lidx, g, x_g):
    # gnn tiles [j, f] bf16 with bias (reuses gsqb's slot after its release)
    bcb_sb = st_["bcb_sb"]
    gnn_t = sb.tile([128, GW], BF16, name=f"gnn_t{lidx}_{g}", tag=f"gnn_t{g}")
    for grp in range(GW // 512):
        gp2 = pp.tile([128, 512], F32, name=f"gp2{lidx}_{g}_{grp}", tag="pz")
        for q in range(4):
            jl = grp * 4 + q
            nc.tensor.matmul(gp2[:, q * 128:(q + 1) * 128],
                             x_g[:, jl * 128:(jl + 1) * 128],
                             w["wgn_bf"][:], start=True, stop=True)
        cs = slice(grp * 512, (grp + 1) * 512)
        nc.vector.tensor_tensor(gnn_t[:, cs], gp2[:], bcb_sb[:], Alu.add)
    st_["gnn_t"][g] = gnn_t


def _mk_prep_plan(nc, sb, pp, st_, w, xg1, plan):
    """plan: list of (emit_jt, token_jt, g, stages) for layer-2 global prep
    emitted inside a chunk loop.  token_jt < emit_jt: a progress token
    written after that jt's sigmoid gates the group's x input so the
    scheduler cannot front-run the prep into the in-order engine queues
    before the corresponding AllGather has landed.  token_jt None = ungated."""
    hooks = {}
    token_jts = set(t for (_, t, _g, _s) in plan if t is not None)
    first_emit = {}
    for (ejt, tjt, g, stages) in plan:
        first_emit[g] = min(first_emit.get(g, 10 ** 9), ejt)

    def mk(ejt, tjt, g, stages):
        def fn(toks):
            if tjt is not None and ejt == first_emit[g]:
                nc.vector.tensor_scalar(xg1[g][:], xg1[g][:], toks[tjt][:],
                                        None, Alu.add)
            _prep_global_group(nc, sb, pp, st_, w, 1, g, xg1[g],
                               use_act=False, stages=stages)
        return fn

    for (ejt, tjt, g, stages) in plan:
        hooks.setdefault(ejt, []).append(mk(ejt, tjt, g, stages))
    return {"hooks": hooks, "token_jts": token_jts}


def _layer_chunk(nc, sb, zp, mp, pp, st_, w, relu, lidx, ic, mid_cb=None):
    """N^2 work + normalize for one i-chunk. Returns xn chunk [128, ICH].

    PE emission is software-pipelined: z(jt+1) is emitted before msg(jt) so
    the in-order PE queue never head-of-line blocks the next z behind a msg
    matmul that waits on sigmoid(jt)."""
    aug_mov = st_["aug_mov"][ic]
    hooks = mid_cb["hooks"] if mid_cb else {}
    token_jts = mid_cb["token_jts"] if mid_cb else set()

    msgp = mp.tile([128, ICH], F32, name=f"msgp{lidx}_{ic}", tag="msg")
    racc = sb.tile([128, ICH], BF16, name=f"racc{lidx}_{ic}", tag="racc",
                   bufs=2)
    toks = {}

    def emit_z(jt):
        g, jl = jt // GJT, jt % GJT
        aug_g = st_["aug_g"][g]
        js = slice(jl * 128, (jl + 1) * 128)
        z = zp.tile([128, ICH], F32, name=f"z{lidx}_{ic}_{jt}", tag="z")
        for h in range(ICH // 512):
            nc.tensor.matmul(z[:, h * 512:(h + 1) * 512], aug_g[:, js],
                             aug_mov[:, h * 512:(h + 1) * 512],
                             start=True, stop=True)
        return z

    # z one-ahead software pipeline: z(jt+1) is emitted before msg(jt) so
    # the in-order PE queue never blocks the next z behind a msg matmul
    # waiting on sigmoid(jt).  (A two-ahead variant was ~24us faster in the
    # cost model but intermittently corrupted results on hardware -- three
    # z tiles in flight over the two-slot PSUM pool.)
    zq = [emit_z(0)]
    for jt in range(JT):
        for fn in hooks.get(jt, []):
            fn(toks)
        g, jl = jt // GJT, jt % GJT
        sqb, gnn_t = st_["sqb"][g], st_["gnn_t"][g]
        js = slice(jl * 128, (jl + 1) * 128)
        adj = sb.tile([128, ICH], BF16, name=f"adj{lidx}_{ic}_{jt}",
                      tag="adj", bufs=6)
        nc.scalar.activation(adj[:], zq[0][:], Act.Sigmoid,
                             bias=sqb[:, jl:jl + 1], scale=1.0)
        zq.pop(0)
        if jt in token_jts:
            tok = sb.tile([128, 1], F32, name=f"tok{lidx}_{ic}_{jt}",
                          tag="tok", bufs=4)
            nc.vector.tensor_scalar_mul(tok[:], adj[:, 0:1], 0.0)
            toks[jt] = tok
        if jt + 1 < JT:
            zq.append(emit_z(jt + 1))
        for h in range(ICH // 512):
            hs = slice(h * 512, (h + 1) * 512)
            nc.tensor.matmul(msgp[:, hs], gnn_t[:, js], adj[:, hs],
                             start=(jt == 0), stop=(jt == JT - 1))
        if jt == 0:
            nc.vector.tensor_copy(racc[:], adj[:])
        else:
            nc.vector.tensor_tensor(racc[:], racc[:], adj[:], Alu.add)

    # normalize phase 1 (quick): xn = msg + corr releases the
    # single-buffered msg PSUM tile as early as possible, and the racc
    # collapse feeds the reciprocal.  Phase 2 (rcp broadcast + multiply) is
    # returned as a closure so the caller can defer it into the next
    # chunk's loop -- otherwise its bc matmuls sit ahead of the next
    # chunk's z prologue in the in-order PE queue, blocked behind the
    # reciprocal chain.
    corr = st_["corr"][ic]
    xn = sb.tile([128, ICH], F32, name=f"xn{lidx}_{ic}", tag=f"xn{lidx}_{ic}")
    for h in range(ICH // 512):
        hs512 = slice(h * 512, (h + 1) * 512)
        nc.vector.tensor_tensor(xn[:, hs512], msgp[:, hs512], corr[:, hs512],
                                Alu.add)

    # collapse the 128 partitions of racc with a ones matmul (bf16 in)
    rsum = sb.tile([1, ICH], F32, name=f"rsum{lidx}_{ic}", tag="rsum", bufs=2)
    for h in range(ICH // 256):
        hs = slice(h * 256, (h + 1) * 256)
        rs = pp.tile([1, 256], F32, name=f"rs{lidx}_{ic}_{h}", tag="pz")
        nc.tensor.matmul(rs[:], w["ones128b"][:], racc[:, hs],
                         start=True, stop=True)
        nc.vector.tensor_copy(rsum[0:1, hs], rs[:])
    rcp = sb.tile([1, ICH], F32, name=f"rcp{lidx}_{ic}", tag="rcp", bufs=2)
    nc.vector.reciprocal(rcp[:], rsum[0:1, :])

    def finish():
        # xn = [relu] (xn * rcp_broadcast)
        for h in range(ICH // 512):
            hs512 = slice(h * 512, (h + 1) * 512)
            bc = pp.tile([128, 512], F32, name=f"bc{lidx}_{ic}_{h}", tag="pz")
            for q in range(2):
                nc.tensor.matmul(
                    bc[:, q * 256:(q + 1) * 256], w["ones1f"][:],
                    rcp[0:1, h * 512 + q * 256: h * 512 + (q + 1) * 256],
                    start=True, stop=True)
            nc.vector.tensor_tensor(xn[:, hs512], xn[:, hs512], bc[:],
                                    Alu.mult)
            if relu:
                nc.vector.tensor_scalar_max(xn[:, hs512], xn[:, hs512], 0.0)
    return xn, finish


def build():
    nc = bacc.Bacc("TRN2", target_bir_lowering=False, debug=False,
                   num_devices=NCORES)

    ins = {}

    def di(name, shape, dt):
        ins[name] = nc.dram_tensor(name, shape, dt, kind="ExternalInput")
        return ins[name]

    di("x_bf", [D, N], BF16)
    di("xr_bf", [D, B], BF16)
    di("xr_f32", [D, B], F32)
    di("ident", [128, 128], BF16)
    di("ones2", [2, GW], BF16)
    for l in range(2):
        di(f"wgl{l}", [D, L], BF16)
        di(f"glb{l}", [L, 1], F32)
        di(f"wgn{l}", [D, D], BF16)
        di(f"wgn32_{l}", [D, D], F32)
        di(f"wgnb{l}", [D, 1], F32)
        di(f"gnnbrow{l}", [1, D], F32)
    di("out_w", [D, NOUT], F32)
    di("out_b", [1, NOUT], F32)
    di("consts", [128, 8], F32)
    y_ext = nc.dram_tensor("y", [B, NOUT], F32, kind="ExternalOutput")

    with tile.TileContext(nc) as tc:
        with (
            tc.tile_pool(name="sb", bufs=1) as sb,
            tc.tile_pool(name="sbl", bufs=2) as sbl,
            tc.tile_pool(name="zp", bufs=2, space="PSUM") as zp,
            tc.tile_pool(name="mp", bufs=1, space="PSUM") as mp,
            tc.tile_pool(name="dram", bufs=1, space="DRAM") as dram,
        ):
            pp = zp     # prep/small psum tiles share the z slots

            def ld(name, shape, dt, pool=sb):
                t = pool.tile(shape, dt, name=f"{name}_sb")
                nc.sync.dma_start(t[:], ins[name][:, :])
                return t

            # --- DMA order: by first use.  bcb (the first PE op) needs
            # gnnbrow0; gp3 needs wgl0 + xr; sqb needs consts.  corr /
            # layer-2 inputs come later.
            wsh = {}
            wl = [dict(), dict()]
            wl[0]["wgl_bf"] = ld("wgl0", [D, L], BF16)
            glb0 = sb.tile([64, 1], F32, name="glb0_sb")
            nc.sync.dma_start(glb0[:], ins["glb0"][:, :])
            wl[0]["glb"] = glb0
            xr_bf0, xr_f0 = [], []
            for ic in range(NIC):
                t = sb.tile([D, ICH], BF16, name=f"xr_bf0_{ic}",
                            tag=f"xr_bf_{ic}")
                nc.sync.dma_start(t[:], ins["xr_bf"][:, ic * ICH:(ic + 1) * ICH])
                xr_bf0.append(t)
            wl[0]["gnnbrow"] = ld("gnnbrow0", [1, D], F32)
            consts_sb = ld("consts", [128, 8], F32)
            for k, nm in enumerate(("negt", "thv", "twot", "sigthv",
                                    "bfsigthv")):
                cv = sb.tile([128, 1], F32, name=f"{nm}_sb")
                nc.vector.tensor_copy(cv[:], consts_sb[:, k:k + 1])
                wsh[nm] = cv
            # aug_g layer-1 group tiles + their ones-row DMAs go early: the
            # first z matmul reads the full aug_g tile including rows 64:66
            wsh["ones2"] = ins["ones2"]
            st0 = {"xr_bf": xr_bf0, "xr_f32": xr_f0}
            for g in range(NG):
                _prep_global_alloc(nc, sb, st0, wsh, 0, g)

            # --- layer-1 x groups (they gate the global prep chain)
            xg0 = []
            for g in range(NG):
                t = sb.tile([D, GW], BF16, name=f"xg0_{g}", tag=f"xg{g}")
                nc.sync.dma_start(t[:], ins["x_bf"][:, g * GW:(g + 1) * GW])
                xg0.append(t)
            wl[0]["wgn_bf"] = ld("wgn0", [D, D], BF16)
            for ic in range(NIC):
                t = sb.tile([D, ICH], F32, name=f"xr_f0_{ic}",
                            tag=f"xr_f_{ic}")
                nc.sync.dma_start(t[:], ins["xr_f32"][:, ic * ICH:(ic + 1) * ICH])
                xr_f0.append(t)
            wl[0]["wgn_f32"] = ld("wgn32_0", [D, D], F32)
            wl[0]["wgnb"] = ld("wgnb0", [D, 1], F32)
            wsh["ident"] = ld("ident", [128, 128], BF16)

            ones64f = sb.tile([64, 1], F32, name="ones64f")
            nc.vector.memset(ones64f[:], 1.0)
            # prime the sigmoid ACT table set while DMAs are in flight
            dummy = sb.tile([1, 1], F32, name="dummy_sig")
            nc.scalar.activation(dummy[:], ones64f[0:1, :], Act.Sigmoid)
            ones1f = sb.tile([1, 128], F32, name="ones1f")
            nc.vector.memset(ones1f[:], 1.0)
            ones64b = sb.tile([64, 1], BF16, name="ones64b")
            nc.vector.memset(ones64b[:], 1.0)
            ones128b = sb.tile([128, 1], BF16, name="ones128b")
            nc.vector.memset(ones128b[:], 1.0)
            wsh["ones64f"] = ones64f
            wsh["ones1f"] = ones1f
            wsh["ones64b"] = ones64b
            wsh["ones128b"] = ones128b
            wsh["ones2"] = ins["ones2"]
            for l in range(2):
                wl[l].update(wsh)

            # ================= layer 1 =================
            # critical-path prep only: chunk-0 aug_mov + group-0 globals.
            # Everything else is spread into the chunk-0 loop via hooks so
            # ~200 prep matmuls don't sit ahead of the first z matmul in
            # the in-order PE queue.
            _prep_local_aug(nc, sb, pp, st0, wl[0], 0, 0)
            _prep_global_group(nc, sb, pp, st0, wl[0], 0, 0, xg0[0])

            def _l1_hook(fn, *a, **k):
                return lambda toks: fn(nc, sb, pp, st0, wl[0], 0, *a, **k)

            l1_hooks = {
                4: [_l1_hook(_prep_local_aug, 1, use_act=False)],
                6: [_l1_hook(_prep_global_group, 1, xg0[1], use_act=True,
                             stages=(0,))],
                12: [_l1_hook(_prep_global_group, 1, xg0[1], use_act=True,
                              stages=(1,))],
                28: [_l1_hook(_prep_local_corr, 0, half=0)],
                36: [_l1_hook(_prep_local_corr, 0, half=1)],
                40: [_l1_hook(_prep_local_corr, 1, half=0)],
                48: [_l1_hook(_prep_local_corr, 1, half=1)],
                48: [_l1_hook(_prep_global_group, 2, xg0[2], use_act=True,
                              stages=(0,))],
                56: [_l1_hook(_prep_global_group, 2, xg0[2], use_act=True,
                              stages=(1,))],
                78: [_l1_hook(_prep_global_group, 3, xg0[3], use_act=False,
                              stages=(0,))],
                86: [_l1_hook(_prep_global_group, 3, xg0[3], use_act=False,
                              stages=(1,))],
            }

            # --- layer-2 + head weights (needed much later)
            wl[1]["wgl_bf"] = ld("wgl1", [D, L], BF16)
            glb1 = sb.tile([64, 1], F32, name="glb1_sb")
            nc.sync.dma_start(glb1[:], ins["glb1"][:, :])
            wl[1]["glb"] = glb1
            wl[1]["wgn_f32"] = ld("wgn32_1", [D, D], F32)
            wl[1]["wgn_bf"] = ld("wgn1", [D, D], BF16)
            wl[1]["wgnb"] = ld("wgnb1", [D, 1], F32)
            wl[1]["gnnbrow"] = ld("gnnbrow1", [1, D], F32)
            out_w_sb = ld("out_w", [D, NOUT], F32)
            out_b_sb = ld("out_b", [1, NOUT], F32)

            x1c_bf = []
            ag_outs = []
            st1 = {}
            for ic in range(NIC):
                if ic == 0:
                    mid = {"hooks": l1_hooks, "token_jts": set()}
                else:
                    mid = _mk_prep_plan(
                        nc, sb, pp, st1, wl[1], st1["xg1"],
                        [(72, 68, 0, (0,)), (80, 68, 0, (1,)),
                         (88, 84, 1, (0,)), (96, 84, 1, (1,))])
                xn, fin_cur = _layer_chunk(nc, sb, zp, mp, pp, st0,
                                           wl[0], True, 0, ic, mid_cb=mid)
                # the finish (scale+relu) must precede the xn consumers
                # emitted in this chunk's tail (AllGather copy, layer-2
                # local prep) -- deferring it would bind their reads to the
                # unscaled intermediate
                fin_cur()
                xbf = sb.tile([D, ICH], BF16, name=f"x1bf_{ic}",
                              tag=f"x1bf_{ic}")
                nc.vector.tensor_copy(xbf[:], xn[:])
                x1c_bf.append(xbf)
                st0[f"xn{ic}"] = xn
                # chunk 0: one AllGather (fully hidden under chunk 1).
                # chunk 1: two half-width AllGathers so the first 4096
                # gathered columns arrive ~25us earlier (they gate the
                # second half of layer-2 chunk 0).
                halves = [slice(0, ICH)] if ic == 0 else [
                    slice(0, ICH // 2), slice(ICH // 2, ICH)]
                for hf, hsl in enumerate(halves):
                    hw = hsl.stop - hsl.start
                    ag_in = dram.tile([D, hw], BF16, name=f"ag_in_{ic}_{hf}")
                    ag_out = dram.tile([NCORES * D, hw], BF16,
                                       name=f"ag_out_{ic}_{hf}",
                                       addr_space="Shared")
                    nc.sync.dma_start(ag_in[:], xbf[:, hsl])
                    nc.gpsimd.collective_compute(
                        "AllGather", Alu.bypass,
                        ins=[ag_in.opt()],
                        outs=[ag_out.opt()],
                        replica_groups=[list(range(NCORES))],
                    )
                    ag_outs.append(ag_out)
                # layer-2 local prep for this chunk (runs during next chunk /
                # the AllGather)
                if ic == 0:
                    st1.update({"xr_bf": x1c_bf, "xr_f32": [st0["xn0"]]})
                else:
                    st1["xr_f32"] = [st0["xn0"], st0["xn1"]]
                _prep_local_aug(nc, sb, pp, st1, wl[1], 1, ic, use_act=False)
                _prep_local_corr(nc, sb, pp, st1, wl[1], 1, ic)
                # layer-2 gathered x groups (ic-major j order).  Only the
                # DMAs are emitted here (early SP-queue position); the prep
                # compute is emitted inside later chunk loops so the
                # in-order engine queues never stall on the collective.
                xg1 = st1.setdefault("xg1", [])
                if ic == 0:
                    # g0 = cores 0..3, g1 = cores 4..7 of ag_out (full width)
                    for half in range(2):
                        g = 2 * ic + half
                        _prep_global_alloc(nc, sb, st1, wl[1], 1, g)
                        t = sb.tile([D, GW], BF16, name=f"xg1_{g}",
                                    tag=f"xg{g}")
                        agsrc = ag_outs[0][half * 4 * D:(half + 1) * 4 * D, :]
                        nc.sync.dma_start(
                            t[:].rearrange("p (r w) -> p r w", r=4),
                            agsrc.rearrange("(r p) w -> p r w", p=D))
                        xg1.append(t)
                else:
                    # g2 = AG#1a (all 8 cores, 512 cols), g3 = AG#1b
                    for hf in range(2):
                        g = 2 + hf
                        _prep_global_alloc(nc, sb, st1, wl[1], 1, g)
                        t = sb.tile([D, GW], BF16, name=f"xg1_{g}",
                                    tag=f"xg{g}")
                        agsrc = ag_outs[1 + hf]
                        nc.sync.dma_start(
                            t[:].rearrange("p (r w) -> p r w", r=8),
                            agsrc.rearrange("(r p) w -> p r w", p=D))
                        xg1.append(t)

            # ================= layer 2 =================
            x2c = {}
            for ic in range(NIC):
                if ic == 0:
                    mid = _mk_prep_plan(
                        nc, sb, pp, st1, wl[1], xg1,
                        [(52, 48, 2, (1,)), (58, 48, 2, (0,)),
                         (84, 80, 3, (1,)), (90, 80, 3, (0,))])
                else:
                    mid = {"hooks": {}, "token_jts": set()}
                x2, fin_cur = _layer_chunk(nc, sb, zp, mp, pp, st1, wl[1],
                                           False, 1, ic, mid_cb=mid)
                fin_cur()
                x2c[ic] = x2

            # ---- output head: softmax(x2 @ out_w + out_b), per chunk so
            # chunk-0's head hides under chunk-1 (exp via sigmoid table:
            # e^x = s/(1-s) -- no ACT table switch)
            for ic in range(NIC):
                NTC = ICH // 128       # 8 row-tiles per chunk
                lg = sbl.tile([128, NTC * NOUT], F32, name=f"lg_{ic}",
                              tag="e")
                for grp in range(ICH // 512):
                    lgp = pp.tile([128, 4 * NOUT], F32, name=f"lg{ic}_{grp}",
                                  tag="pz")
                    for q in range(4):
                        it = grp * 4 + q
                        qs = slice(q * NOUT, (q + 1) * NOUT)
                        nc.tensor.matmul(lgp[:, qs], ones1f[:], out_b_sb[:, :],
                                         start=True, stop=False)
                        nc.tensor.matmul(lgp[:, qs],
                                         x2c[ic][:, it * 128:(it + 1) * 128],
                                         out_w_sb[:], start=False, stop=True)
                    gof = grp * 4 * NOUT
                    nc.vector.tensor_copy(lg[:, gof:gof + 4 * NOUT], lgp[:])
                sg = sbl.tile([128, NTC * NOUT], F32, name=f"sg_{ic}",
                              tag="yt")
                nc.scalar.activation(sg[:], lg[:], Act.Sigmoid)
                om = sbl.tile([128, NTC * NOUT], F32, name=f"om_{ic}",
                              tag="om")
                nc.vector.tensor_scalar(om[:], sg[:], -1.0, 1.0, Alu.mult,
                                        Alu.add)
                rom = sbl.tile([128, NTC * NOUT], F32, name=f"rom_{ic}",
                               tag="om")
                nc.vector.reciprocal(rom[:], om[:])
                e = sbl.tile([128, NTC * NOUT], F32, name=f"e_{ic}", tag="yt")
                nc.vector.tensor_tensor(e[:], sg[:], rom[:], Alu.mult)
                e3 = e[:].rearrange("p (q n) -> p q n", n=NOUT)
                es = sbl.tile([128, NTC], F32, name=f"es_{ic}", tag="es")
                nc.vector.reduce_sum(es[:], e3, axis=AX)
                rse = sbl.tile([128, NTC], F32, name=f"rse_{ic}", tag="rse")
                nc.vector.reciprocal(rse[:], es[:])
                yt = sbl.tile([128, NTC * NOUT], F32, name=f"yt_{ic}",
                              tag="e")
                nc.vector.tensor_tensor(
                    yt[:].rearrange("p (q n) -> p q n", n=NOUT), e3,
                    rse[:].rearrange("p q -> p q ()").broadcast_to(
                        [128, NTC, NOUT]),
                    Alu.mult)
                for itc in range(NTC):
                    it = ic * NTC + itc
                    nc.sync.dma_start(y_ext[it * 128:(it + 1) * 128, :],
                                      yt[:, itc * NOUT:(itc + 1) * NOUT])

    nc.compile()
    return nc


def _get_nc():
    if "nc" not in _NC_CACHE:
        _NC_CACHE["nc"] = build()
    return _NC_CACHE["nc"]


def kernel(feat_matrix, gl_w0, gl_b0, gl_w1, gl_b1,
           gnn_w0, gnn_b0, gnn_w1, gnn_b1,
           out_w, out_b, temp, theta,
           adj_matrix=None, get_item_index=None, set_index=None,
           val_index=None, mask_matrix=None, **_unused):
    bf = ml_dtypes.bfloat16
    f32 = np.float32

    x = np.ascontiguousarray(np.asarray(feat_matrix, dtype=f32))
    assert x.shape == (N, D)
    t = 1.0 + float(np.asarray(temp))
    th = 5.0 + float(np.asarray(theta))
    sigth = float(1.0 / (1.0 + np.exp(-np.float32(th))))
    bfsigth = float(np.float32(bf(np.float32(sigth))))
    lo16 = float(np.float32(bf(np.nextafter(np.float32(sigth), np.float32(0.0)))))
    hi16 = float(np.float32(bf(np.nextafter(np.float32(sigth), np.float32(1.0)))))
    assert lo16 == bfsigth == hi16, "sigth too close to a bf16 boundary"

    xT = np.ascontiguousarray(x.T)                       # [D, N] f32
    xT_bf = xT.astype(bf)

    def colvec(v):
        return np.full((128, 1), v, dtype=f32)

    common = {
        "x_bf": xT_bf,
        "ident": np.eye(128, dtype=bf),
        "ones2": np.ones((2, GW), dtype=bf),
        "out_w": np.ascontiguousarray(np.asarray(out_w, dtype=f32)),
        "out_b": np.asarray(out_b, dtype=f32).reshape(1, NOUT),
        "consts": np.concatenate(
            [colvec(-t), colvec(th), colvec(2.0 * t), colvec(sigth),
             colvec(bfsigth), colvec(0.0), colvec(0.0), colvec(0.0)],
            axis=1),
    }
    for l, (wgl, glb, wgn, gnb) in enumerate(
            [(gl_w0, gl_b0, gnn_w0, gnn_b0), (gl_w1, gl_b1, gnn_w1, gnn_b1)]):
        wgl = np.ascontiguousarray(np.asarray(wgl, dtype=f32))
        wgn = np.ascontiguousarray(np.asarray(wgn, dtype=f32))
        common[f"wgl{l}"] = wgl.astype(bf)
        common[f"glb{l}"] = np.asarray(glb, dtype=f32).reshape(L, 1)
        common[f"wgn{l}"] = wgn.astype(bf)
        common[f"wgn32_{l}"] = wgn
        common[f"wgnb{l}"] = np.asarray(gnb, dtype=f32).reshape(D, 1)
        common[f"gnnbrow{l}"] = np.asarray(gnb, dtype=f32).reshape(1, D)

    in_maps = []
    for c in range(NCORES):
        blk = slice(c * B, (c + 1) * B)
        m = dict(common)
        m["xr_bf"] = np.ascontiguousarray(xT_bf[:, blk])
        m["xr_f32"] = np.ascontiguousarray(xT[:, blk])
        in_maps.append(m)

    nc = _get_nc()
    res = run_bass_kernel_spmd(nc, in_maps, core_ids=list(range(NCORES)))
    return np.concatenate([res.results[c]["y"] for c in range(NCORES)], axis=0)


if __name__ == "__main__":
    import time
    t0 = time.time()
    nc = build()
    print(f"build+compile: {time.time() - t0:.1f}s")
